# revision 11
# baseline (speedup 1.0000x reference)
"""Cubic B-spline FFD 3D upsampling kernel for Trainium2 (8 NeuronCores).

Reference: v [4,3,44,52,44] f32 -> out [4,3,160,192,160] f32 via three separable
stride-4 transposed convs (cubic B-spline, len 15) + crop [4:4+D].

Sharding: output z (160) split into 8 chunks of 20; core c consumes input
z-planes [5c, 5c+8) and writes out[:, :, 20c:20c+20].

Per-core pipeline (fp16 activations, f32 PSUM):
  L0 [128=(g*64 + y52), (b6, zi8, xi44)]  <- one contiguous DMA (host pre-layouts)
  z-pass (phases r=1,2,3): DVE polyphase MACs split as
     tensor_scalar_mul (4x mode) + tensor_tensor add (2x mode) -> L1
  z-phase r=0 is folded into the y-pass as PE accumulation over 3 taps with
     pre-scaled wy weights (reads L0 directly).
  y-pass: per (g, b-pair, zo): matmul lhsT=[y52,(b2,xi44)=88] rhs=wy[y,192]
     -> psum [88, 192];  4 zo per 2-bank psum tile
  y-copy: psum -> L2 [88=(b2,xi44), m=(zo20,yo192)=3840] fp16
  x-pass: chunk j: lhsT = L2[44bm:+44, m=j::30 (128)] @ wx[44,160] -> psum
  x-copy: psum [128, 2x480] -> st [128, (r30, xo160)] fp16 (partition p holds
     output rows 30p..30p+30 of the (zo,yo) raster -> 9600B contiguous lines)
  out-DMA: 2 per bc, [128, 1920/2880] -> HBM, fp16 (host upcasts to f32)

Copies are spread across DVE / Act / GPSIMD to balance engine busy time.
"""

import numpy as np

N_CORES = 8
ZIN, YIN, XIN = 44, 52, 44
ZOUT, YOUT, XOUT = 160, 192, 160
ZI = 8              # input z planes per core
ZSH = 20            # output z per core
B6 = 6              # batch-channels per partition group
M_TOT = ZSH * YOUT  # 3840 output rows per bc
XP = 64             # xi padded to 64 (PE base-partition constraint: 0/32/64)
NCH = 30            # x-pass chunks per bc (M_TOT / 128)

# --- tuning knobs ---
# z phases folded into the y-pass as PE accumulation, per group index 0..5
PE_OFFLOAD = {0: (0, 1, 2, 3), 1: (0, 2), 2: (0, 2), 3: (0, 2), 4: (0, 2), 5: (0, 2)}
# per-group engine for remaining z MACs: "dve" = mul+add on DVE only,
# "hybrid" = DVE muls + gpsimd pair-adds + DVE final add
Z_MODE = {0: "dve", 1: "hybrid", 2: "hybrid", 3: "hybrid", 4: "hybrid", 5: "hybrid"}
# weighted engine choice for PSUM->SBUF copies: (dve, act)
COPY_W = (45.0, 55.0)


def _bspline_kernel():
    x = (np.arange(15) - 7) / 4.0
    t = np.abs(x)
    return np.where(
        t < 1.0, 2.0 / 3.0 + (0.5 * t - 1.0) * t**2,
        np.where(t < 2.0, ((2.0 - t) ** 3) / 6.0, 0.0)
    ).astype(np.float32)


_W = _bspline_kernel()


def _exp_mat(n_in, n_out):
    """M[i, o] = weight of control point i on (post-crop) output o."""
    M = np.zeros((n_in, n_out), dtype=np.float32)
    for o in range(n_out):
        ilo = int(np.ceil((o - 3) / 4))
        ihi = (o + 11) // 4
        for i in range(max(ilo, 0), min(ihi, n_in - 1) + 1):
            n = 4 * i - o + 3
            if 0 <= n < 15:
                M[i, o] = _W[n]
    return M


def _ztaps():
    """Per phase r: list of (tap t, weight); input plane = k + t for zo=4k+r."""
    out = []
    for r in range(4):
        taps = []
        for t in range(4):
            n = 4 * t + 3 - r
            if 0 <= n < 15:
                taps.append((t, float(_W[n])))
        out.append(taps)
    return out


_ZTAPS = _ztaps()
_NC_CACHE = {}


def _build_nc():
    import concourse.bacc as bacc
    import concourse.mybir as mybir
    from concourse.tile import TileContext

    FP32 = mybir.dt.float32
    FP16 = mybir.dt.float16
    ADD = mybir.AluOpType.add
    MULT = mybir.AluOpType.mult

    nc = bacc.Bacc()
    v = nc.declare_dram_parameter("v", [128, B6 * ZI * XP], FP16, isOutput=False)
    wy = nc.declare_dram_parameter("wy", [128, YOUT], FP16, isOutput=False)
    nt = [len(_ZTAPS[r]) for r in range(4)]
    wyr = [nc.declare_dram_parameter(f"wyr{r}", [128, nt[r] * YOUT], FP16,
                                     isOutput=False) for r in range(4)]
    wx = nc.declare_dram_parameter("wx", [128, XOUT], FP16, isOutput=False)
    out = nc.declare_dram_parameter("out", [12, M_TOT, XOUT], FP16, isOutput=True)

    with TileContext(nc) as tc:
        with (
            tc.tile_pool(name="const", bufs=1) as cpool,
            tc.tile_pool(name="io", bufs=1) as iopool,
            tc.tile_pool(name="tmp", bufs=4) as tmppool,
            tc.tile_pool(name="l2", bufs=3) as l2pool,
            tc.tile_pool(name="stp", bufs=3) as stpool,
            tc.tile_pool(name="psy", bufs=2, space="PSUM") as psyp,
            tc.tile_pool(name="psx", bufs=2, space="PSUM") as psxp,
        ):
            wyt = cpool.tile([128, YOUT], FP16)
            nc.sync.dma_start(out=wyt[:, :], in_=wy[:, :])
            wyrv = []
            for r in range(4):
                wt = cpool.tile([128, nt[r] * YOUT], FP16, name=f"wyr{r}t")
                nc.sync.dma_start(out=wt[:, :], in_=wyr[r][:, :])
                wyrv.append(wt.rearrange("p (t y) -> p t y", t=nt[r]))
            wxt = cpool.tile([128, XOUT], FP16)
            nc.sync.dma_start(out=wxt[:, :], in_=wx[:, :])

            warm = psxp.tile([128, 1024], FP32, name="psx")
            for i in range(8):
                nc.tensor.matmul(warm[:, 0:192], lhsT=wyt[0:52, 0:128],
                                 rhs=wyt[0:52, :], start=True, stop=True)

            L0 = iopool.tile([128, B6 * ZI * XP], FP16)
            for g in range(2):
                nc.sync.dma_start(out=L0[64 * g:64 * g + 64, :],
                                  in_=v[64 * g:64 * g + 64, :])
            L0v = L0.rearrange("p (z b x) -> p z b x", z=ZI, b=B6)

            L1 = iopool.tile([128, B6 * ZSH * XP], FP16)
            L1v = L1.rearrange("p (k r b x) -> p k r b x", k=5, r=4, b=B6)

            groups = [(g, bp) for g in range(2) for bp in range(3)]

            xw = list(COPY_W)
            xacc = [0.0, 0.0]

            def pick():
                tot = sum(xw)
                for k in range(2):
                    xacc[k] += xw[k] / tot
                i = max(range(2), key=lambda k: xacc[k])
                xacc[i] -= 1.0
                return i

            def copy_ps(dst, src):
                if pick() == 0:
                    nc.vector.tensor_copy(out=dst, in_=src)
                else:
                    nc.scalar.copy(dst, src)

            def emit_z(gi):
                """z-pass phases r not PE-offloaded, for group index gi."""
                g, bp = groups[gi]
                lo, hi = 64 * g, 64 * g + YIN
                for r in range(4):
                    if r in PE_OFFLOAD[gi]:
                        continue
                    dst = L1v[lo:hi, :, r, 2 * bp:2 * bp + 2, 0:XIN]
                    taps = _ZTAPS[r]
                    t0, w0 = taps[0]
                    if Z_MODE[gi] == "hybrid" and len(taps) == 4:
                        # DVE scaled muls (4x mode) into contiguous tmps; gpsimd
                        # adds tmp pairs (contiguous (b,x) merges to 3D); DVE
                        # does the final strided add into L1.
                        tms = []
                        for t, w in taps:
                            tm = tmppool.tile([128, 2 * 5 * XIN], FP16)
                            tmv = tm.rearrange("p (k b x) -> p k b x", k=5, b=2)
                            nc.vector.tensor_scalar_mul(
                                tmv[lo:hi], L0v[lo:hi, t:t + 5, 2 * bp:2 * bp + 2, 0:XIN], w)
                            tms.append(tm)
                        pa = tmppool.tile([128, 2 * 5 * XIN], FP16)
                        pb = tmppool.tile([128, 2 * 5 * XIN], FP16)
                        nc.gpsimd.tensor_tensor(
                            out=pa[lo:hi, :], in0=tms[0][lo:hi, :],
                            in1=tms[1][lo:hi, :], op=ADD)
                        nc.gpsimd.tensor_tensor(
                            out=pb[lo:hi, :], in0=tms[2][lo:hi, :],
                            in1=tms[3][lo:hi, :], op=ADD)
                        nc.vector.tensor_tensor(
                            out=dst, in0=pa.rearrange("p (k b x) -> p k b x", k=5, b=2)[lo:hi],
                            in1=pb.rearrange("p (k b x) -> p k b x", k=5, b=2)[lo:hi], op=ADD)
                    else:
                        nc.vector.tensor_scalar_mul(
                            dst, L0v[lo:hi, t0:t0 + 5, 2 * bp:2 * bp + 2, 0:XIN], w0)
                        for t, w in taps[1:]:
                            tm = tmppool.tile([128, 2 * 5 * XIN], FP16)
                            tmv = tm.rearrange("p (k b x) -> p k b x", k=5, b=2)
                            nc.vector.tensor_scalar_mul(
                                tmv[lo:hi], L0v[lo:hi, t:t + 5, 2 * bp:2 * bp + 2, 0:XIN], w)
                            nc.vector.tensor_tensor(
                                out=dst, in0=dst, in1=tmv[lo:hi], op=ADD)

            def emit_y(gi):
                """y-pass for group: 5 psum quads of 4 zo each -> L2 tile."""
                g, bp = groups[gi]
                lo, hi = 64 * g, 64 * g + YIN
                L2g = l2pool.tile([128, M_TOT], FP16)
                offs = (0, 192, 512, 704)
                for q in range(5):
                    psy = psyp.tile([128, 1024], FP32)
                    for s in range(4):
                        zo, r, off = 4 * q + s, s, offs[s]
                        if r in PE_OFFLOAD[gi]:
                            taps = _ZTAPS[r]
                            for i, (t, w) in enumerate(taps):
                                nc.tensor.matmul(
                                    psy[:, off:off + YOUT],
                                    lhsT=L0v[lo:hi, q + t, 2 * bp:2 * bp + 2, :],
                                    rhs=wyrv[r][lo:hi, i, :],
                                    start=(i == 0), stop=(i == len(taps) - 1),
                                )
                        else:
                            nc.tensor.matmul(
                                psy[:, off:off + YOUT],
                                lhsT=L1v[lo:hi, q, r, 2 * bp:2 * bp + 2, :],
                                rhs=wyt[lo:hi, :],
                                start=True, stop=True,
                            )
                    psyv = psy.rearrange("p (h x) -> p h x", h=2)
                    copy_ps(
                        L2g.rearrange("p (h x) -> p h x", h=10)[:, 2 * q:2 * q + 2, :],
                        psyv[:, :, 0:2 * YOUT])
                return L2g

            def emit_x(gi, L2g):
                """x-pass + staging + out DMA for the 2 bc of this group."""
                g, bp = groups[gi]  # noqa
                L2j = L2g.rearrange("p (k j) -> p k j", j=NCH)
                for bm in range(2):
                    bc = 6 * g + 2 * bp + bm
                    st = stpool.tile([128, NCH * XOUT], FP16)
                    for pair in range(5):
                        psx = psxp.tile([128, 1024], FP32)
                        for u in range(6):
                            j = 6 * pair + u
                            off = 512 * (u // 3) + 160 * (u % 3)
                            nc.tensor.matmul(
                                psx[:, off:off + XOUT],
                                lhsT=L2j[64 * bm:64 * bm + XIN, :, j],
                                rhs=wxt[64 * bm:64 * bm + XIN, :],
                                start=True, stop=True,
                            )
                        psxv = psx.rearrange("p (h x) -> p h x", h=2)
                        dst = st.rearrange("p (pr x) -> p pr x", pr=5)[:, pair, :]
                        dstv = dst.rearrange("p (h x) -> p h x", h=2)
                        copy_ps(dstv, psxv[:, :, 0:480])
                        if pair == 1:
                            nc.sync.dma_start(
                                out=out[bc].rearrange("(p r) x -> p (r x)", p=128)[:, 0:1920],
                                in_=st[:, 0:1920])
                    if gi == 5 and bm == 1:
                        nc.sync.dma_start(
                            out=out[bc].rearrange("(p r) x -> p (r x)", p=128)[:, 1920:3840],
                            in_=st[:, 1920:3840])
                        nc.sync.dma_start(
                            out=out[bc].rearrange("(p r) x -> p (r x)", p=128)[:, 3840:4800],
                            in_=st[:, 3840:4800])
                    else:
                        nc.sync.dma_start(
                            out=out[bc].rearrange("(p r) x -> p (r x)", p=128)[:, 1920:4800],
                            in_=st[:, 1920:4800])

            # --- software-pipelined emission: x(k) before y(k+1) on PE ---
            emit_z(0)
            emit_z(1)
            L2s = {0: emit_y(0)}
            for k in range(6):
                if k + 2 < 6:
                    emit_z(k + 2)
                emit_x(k, L2s.pop(k))
                if k + 1 < 6:
                    L2s[k + 1] = emit_y(k + 1)
    nc.compile()
    return nc


def _get_nc():
    if "nc" not in _NC_CACHE:
        _NC_CACHE["nc"] = _build_nc()
    return _NC_CACHE["nc"]


def _host_weights():
    f16 = np.float16
    ey = _exp_mat(YIN, YOUT)
    ex = _exp_mat(XIN, XOUT)
    wy128 = np.zeros((128, YOUT), dtype=np.float32)
    wy128[0:YIN] = ey
    wy128[64:64 + YIN] = ey
    wx128 = np.zeros((128, XOUT), dtype=np.float32)
    wx128[0:XIN] = ex
    wx128[64:64 + XIN] = ex
    def scaled(r):
        taps = _ZTAPS[r]
        m = np.zeros((128, len(taps) * YOUT), dtype=np.float32)
        for i, (t, w) in enumerate(taps):
            m[:, i * YOUT:(i + 1) * YOUT] = wy128 * w
        return m.astype(f16)
    return (wy128.astype(f16), [scaled(r) for r in range(4)],
            wx128.astype(f16))


def kernel(v):
    from concourse.bass_utils import run_bass_kernel_spmd

    f16 = np.float16
    v = np.asarray(v).astype(np.float32).reshape(12, ZIN, YIN, XIN)
    wy_h, wyr_h, wx_h = _host_weights()

    in_maps = []
    for c in range(N_CORES):
        slab = v[:, 5 * c:5 * c + ZI]                      # [12, 8, 52, 44]
        arr = np.zeros((128, ZI, B6, XP), dtype=f16)
        arr[0:YIN, :, :, 0:XIN] = slab[0:6].transpose(2, 1, 0, 3)   # y, z, b, x
        arr[64:64 + YIN, :, :, 0:XIN] = slab[6:12].transpose(2, 1, 0, 3)
        in_maps.append({
            "v": np.ascontiguousarray(arr.reshape(128, B6 * ZI * XP)),
            "wy": wy_h, "wx": wx_h,
            **{f"wyr{r}": wyr_h[r] for r in range(4)},
        })

    nc = _get_nc()
    res = run_bass_kernel_spmd(nc, in_maps, core_ids=list(range(N_CORES)))

    outf = np.empty((12, ZOUT, YOUT, XOUT), dtype=np.float32)
    for c in range(N_CORES):
        blk = res.results[c]["out"]                        # [12, 3840, 160] fp16
        outf[:, ZSH * c:ZSH * (c + 1)] = (
            blk.astype(np.float32).reshape(12, ZSH, YOUT, XOUT))
    return outf.reshape(4, 3, ZOUT, YOUT, XOUT)
